# revision 1
# baseline (speedup 1.0000x reference)
"""ContraNorm Trainium2 kernel (8 NeuronCores, flash-style, no NxN materialization).

Reference computation (N=16384, D=256, f32):
    x_norm = x / max(||x||_row, 1e-12)
    sim    = softmax(x_norm @ x_norm.T, axis=1)
    out    = 1.1 * x - 0.1 * (sim @ x)

Sharding: row-parallel. Every core receives the FULL x plus its own 2048-row
slice xr; core c computes output rows [c*2048, (c+1)*2048). No collectives.

Per-core algorithm (matmul operands fp16, accumulation fp32 in PSUM):
  setup (per 4-chunk group, software-pipelined with the main loop):
    ssq[n]  = sum_d x[n,d]^2                  (ACT Square + accum_out)
    rnorm   = 1/sqrt(ssq)                     (ACT sqrt + DVE reciprocal)
    xa      = [fp16(x) | 1.0]  natural layout (GPSIMD copy + memset)
    xn      = fp16(x) * rnorm                 (DVE tensor_scalar)
    xnT     = transpose(xn)   [256, N]        (DMA xbar transpose on Sync)
  main (flash-style; cosine sims bounded in [-1,1] => no max-subtraction):
    phased over n so PE tracks the transpose stream; for each 8-chunk phase,
    each 512-row m-tile computes
      S_T[n,m] = xnT_chunk.T @ xnTm_tile     (PE, PSUM f32)
      E = exp(S_T)                           (ACT, fp16 out, [128,1024] insts)
      Paug[m, 0:257] += E_sub.T @ [x | 1]    (PE accumulate; col 256 = sum(exp))
    then flushes Paug into SBUF accumulators (DVE adds).
  finalize: out_rows = 1.1*xr - 0.1 * Pacc[:, :256] / Pacc[:, 256]
"""

import numpy as np

N, D, NCORES = 16384, 256, 8
M = N // NCORES          # 2048 rows per core
P = 128                  # partitions
SCALE = 0.1

_NC = None               # cached compiled Bass module


def build(n=N, m=M, compile=True):
    """Build the per-core Bass program for full-row-count n, own-rows m."""
    import concourse.bass as bass
    import concourse.tile as tile
    from concourse import bacc, mybir
    from contextlib import ExitStack

    F16 = mybir.dt.float16
    F32 = mybir.dt.float32
    AF = mybir.ActivationFunctionType

    nch = n // P             # n-chunks
    rch = m // P             # own row-chunks
    mt_w = min(512, m)       # m-tile width (S matmul free dim)
    nmt = m // mt_w          # m-tiles
    msub = mt_w // P         # 128-row subtiles per m-tile
    G = 4                    # chunks per setup group
    ngrp = nch // G
    CP = min(8, nch)         # chunks per main-loop phase
    nph = nch // CP
    GPP = CP // G            # setup groups per phase
    LOOKAHEAD = 2            # phases of setup emitted ahead of compute

    # Force all activations onto the one table set that covers Square/Ln/Exp
    # (the default chooser picks the ln-only set for Ln, causing a ~1.3us
    # ACT table reload per normalization group). Indices must stay aligned
    # with act_info.json, so blank the other sets rather than filtering.
    if not getattr(bacc, "_contranorm_act_patch", False):
        _orig_tables = bacc.get_activation_tables

        def _patched_tables(arch):
            keep = "natural_log_exp_and_others"
            return {k: (v if k == keep else set())
                    for k, v in _orig_tables(arch).items()}

        bacc.get_activation_tables = _patched_tables
        bacc._contranorm_act_patch = True

    nc = bacc.Bacc("TRN2", debug=False, num_devices=NCORES)
    x_d = nc.dram_tensor("x", (n, D), F32, kind="ExternalInput").ap()
    xr_d = nc.dram_tensor("xr", (m, D), F32, kind="ExternalInput").ap()
    out_d = nc.dram_tensor("out", (m, D), F32, kind="ExternalOutput").ap()

    # DRAM views: [partition, chunk, d]
    x_c = x_d.rearrange("(c p) d -> p c d", p=P)
    xr_c = xr_d.rearrange("(c p) d -> p c d", p=P)
    out_c = out_d.rearrange("(c p) d -> p c d", p=P)

    with tile.TileContext(nc) as tc, ExitStack() as ctx:
        big = ctx.enter_context(tc.tile_pool(name="big", bufs=1))
        ld = ctx.enter_context(tc.tile_pool(name="ld", bufs=2))
        sc_p = ctx.enter_context(tc.tile_pool(name="scr", bufs=2))
        xnp = ctx.enter_context(tc.tile_pool(name="xn", bufs=8))
        ep = ctx.enter_context(tc.tile_pool(name="exp", bufs=3))
        fin = ctx.enter_context(tc.tile_pool(name="fin", bufs=2))
        sp = ctx.enter_context(tc.tile_pool(name="spsum", bufs=2, space="PSUM"))
        pp = ctx.enter_context(tc.tile_pool(name="ppsum", bufs=1, space="PSUM"))

        # persistent tiles
        xa = big.tile([P, nch, 257], F16)          # raw x fp16 + ones col
        xnT0 = big.tile([P, n], F16)               # x_norm^T rows d=0..127
        xnT1 = big.tile([P, n], F16)               # x_norm^T rows d=128..255
        xnTm0 = big.tile([P, m], F16)              # core rows, normalized, transposed
        xnTm1 = big.tile([P, m], F16)
        xrs = big.tile([P, rch, D], F32)           # 1.1 * xr
        xrf = big.tile([P, rch, D], F16)           # fp16 copy of xr
        pacc = big.tile([P, nmt * msub, 257], F32)  # SBUF P/sumexp accumulators
        ssq_all = big.tile([P, nch], F32)
        rnorm_all = big.tile([P, nch], F32)
        ssq_r = big.tile([P, rch], F32)
        rnorm_r = big.tile([P, rch], F32)

        def xr_chain():
            """Core's own rows: xrs, xnTm (gates the very first matmul)."""
            for g in range(max(1, rch // G)):
                gw = min(G, rch)
                xt = ld.tile([P, G, D], F32, name=f"xtr{g}", tag="xt")
                nc.sync.dma_start(xt[:, 0:gw, :], xr_c[:, g * gw:(g + 1) * gw, :])
                nc.vector.tensor_scalar_mul(xrs[:, g * gw:(g + 1) * gw, :],
                                            xt[:, 0:gw, :], 1.1)
                for j in range(gw):
                    c = g * gw + j
                    scr = sc_p.tile([P, D], F16, tag="sq_scratch", name=f"scr_r{c}")
                    # row sum-of-squares via ACT Square + accum (TTR/tensor_reduce
                    # are broken on this runtime)
                    nc.scalar.activation(scr[:], xt[:, j, :], AF.Square,
                                         accum_out=ssq_r[:, c:c + 1])
                    nc.gpsimd.tensor_copy(xrf[:, c, :], xt[:, j, :])
                s = sc_p.tile([P, gw], F32, tag="nrm_scratch", name=f"s_r{g}")
                # rnorm = exp(-0.5*ln(ssq)): Ln/Exp/Square/Copy share ONE ACT
                # table set (natural_log_exp_and_others) -> no table reloads
                nc.scalar.activation(s[:], ssq_r[:, g * gw:(g + 1) * gw], AF.Ln)
                nc.scalar.activation(rnorm_r[:, g * gw:(g + 1) * gw], s[:],
                                     AF.Exp, scale=-0.5)
                for j in range(gw):
                    c = g * gw + j
                    xn = xnp.tile([P, D], F16, name=f"xnr{c}", tag="xn")
                    nc.vector.tensor_scalar_mul(xn[:], xrf[:, c, :],
                                                rnorm_r[:, c:c + 1])
                    # ACT is idle this early; give it the one-off xnTm1 set
                    nc.sync.dma_start_transpose(xnTm0[:, c * P:(c + 1) * P],
                                                xn[:, 0:P])
                    nc.scalar.dma_start_transpose(xnTm1[:, c * P:(c + 1) * P],
                                                  xn[:, P:D])

        def setup_piece(g, piece):
            """Setup for chunks 4g..4g+3, split into 4 pieces that the phase
            emitter interleaves between m-tile sections (keeps ACT square
            bursts short so they never stall the exp stream)."""
            if piece == 0:
                xt = ld.tile([P, G, D], F32, name=f"xt{g}", tag="xt")
                nc.gpsimd.dma_start(xt[:], x_c[:, g * G:(g + 1) * G, :])
                for j in range(G):
                    c = g * G + j
                    nc.gpsimd.tensor_copy(xa[:, c, 0:D], xt[:, j, :])
                    nc.gpsimd.memset(xa[:, c, D:257], 1.0)
            elif piece in (1, 2):
                for j in ((0, 1) if piece == 1 else (2, 3)):
                    c = g * G + j
                    scr = sc_p.tile([P, D], F16, tag="sq_scratch", name=f"scr{c}")
                    nc.scalar.activation(scr[:], xa[:, c, 0:D], AF.Square,
                                         accum_out=ssq_all[:, c:c + 1])
                if piece == 2:
                    s = sc_p.tile([P, G], F32, tag="nrm_scratch", name=f"s{g}")
                    nc.scalar.activation(s[:], ssq_all[:, g * G:(g + 1) * G], AF.Ln)
                    nc.scalar.activation(rnorm_all[:, g * G:(g + 1) * G], s[:],
                                         AF.Exp, scale=-0.5)
            else:
                for j in range(G):
                    c = g * G + j
                    xn = xnp.tile([P, D], F16, name=f"xn{c}", tag="xn")
                    nc.vector.tensor_scalar_mul(xn[:], xa[:, c, 0:D],
                                                rnorm_all[:, c:c + 1])
                    nc.sync.dma_start_transpose(xnT0[:, c * P:(c + 1) * P],
                                                xn[:, 0:P])
                    nc.sync.dma_start_transpose(xnT1[:, c * P:(c + 1) * P],
                                                xn[:, P:D])

        def setup_group(g):
            for piece in range(4):
                setup_piece(g, piece)

        def phase(ph, setup_jobs=()):
            """All m-tiles consume chunks [ph*CP, (ph+1)*CP); flush into pacc.

            One-deep software pipeline on PE: emit S-matmuls of iteration i
            before P-matmuls of iteration i-1, so exp(i-1) on ACT hides under
            S(i) instead of stalling the in-order PE queue. setup_jobs are
            (g, piece) items interleaved between the m-tile sections.
            """
            nonlocal pend
            jobs = list(setup_jobs)
            for mt in range(nmt):
                m0 = mt * mt_w
                paug = [pp.tile([P, 257], F32, tag=f"paug{ms}",
                                name=f"paug{ms}_{ph}_{mt}") for ms in range(msub)]
                for scn in range(CP // 2):
                    c0 = ph * CP + scn * 2
                    stp = sp.tile([P, 2, mt_w], F32, name=f"stp{ph}_{mt}_{scn}",
                                  tag="stp")
                    for j in range(2):
                        c = c0 + j
                        nc.tensor.matmul(stp[:, j, :], xnT0[:, c * P:(c + 1) * P],
                                         xnTm0[:, m0:m0 + mt_w],
                                         start=True, stop=False)
                        nc.tensor.matmul(stp[:, j, :], xnT1[:, c * P:(c + 1) * P],
                                         xnTm1[:, m0:m0 + mt_w],
                                         start=False, stop=True)
                    es = ep.tile([P, 2, mt_w], F16, name=f"es{ph}_{mt}_{scn}",
                                 tag="es")
                    nc.scalar.activation(es[:], stp[:], AF.Exp)
                    emit_pending()
                    pend = (es, c0, ph, paug, mt)
                # spread the lookahead setup work across the phase
                nj = (len(jobs) + nmt - 1 - mt) // (nmt - mt) if jobs else 0
                for _ in range(nj):
                    g, piece = jobs.pop(0)
                    setup_piece(g, piece)

        def emit_pending():
            nonlocal pend
            if pend is None:
                return
            es, c0, ph, paug, mt = pend
            pend = None
            for j in range(2):
                c = c0 + j
                first = (c == ph * CP)
                last = (c == ph * CP + CP - 1)
                for ms in range(msub):
                    nc.tensor.matmul(
                        paug[ms][:], es[:, j, ms * P:(ms + 1) * P],
                        xa[:, c, :], start=first, stop=last,
                    )
            if (c0 - ph * CP) // 2 == CP // 2 - 1:
                # last iteration of (ph, mt): flush Paug into the SBUF accs
                for ms in range(msub):
                    acc = pacc[:, mt * msub + ms, :]
                    if ph == 0:
                        nc.vector.tensor_copy(acc, paug[ms][:])
                    else:
                        nc.vector.tensor_add(acc, acc, paug[ms][:])

        def finalize():
            for rc in range(nmt * msub):
                r = fin.tile([P, 1], F32, tag="recip", name=f"r{rc}")
                nc.vector.reciprocal(r[:], pacc[:, rc, 256:257])
                rs = fin.tile([P, 1], F32, tag="rscaled", name=f"rs{rc}")
                nc.vector.tensor_scalar_mul(rs[:], r[:], -SCALE)
                t1 = fin.tile([P, D], F32, tag="scaledP", name=f"t1{rc}")
                nc.vector.tensor_scalar_mul(t1[:], pacc[:, rc, 0:D], rs[:])
                ot = fin.tile([P, D], F32, tag="otile", name=f"ot{rc}")
                nc.vector.tensor_add(ot[:], xrs[:, rc, :], t1[:])
                nc.gpsimd.dma_start(out_c[:, rc, :], ot[:])

        pend = None
        # ---- software-pipelined emission: setup stays LOOKAHEAD phases
        # ahead of compute; its pieces are interleaved inside each phase so
        # no engine sees a long setup burst ----
        xr_chain()
        prefill = min(ngrp, GPP * LOOKAHEAD)
        for g in range(prefill):
            setup_group(g)
        emitted = prefill
        for ph in range(nph):
            want = min(ngrp, GPP * (ph + 1 + LOOKAHEAD))
            jobs = [(g, piece) for g in range(emitted, want) for piece in range(4)]
            emitted = want
            phase(ph, jobs)
        emit_pending()
        finalize()

    if compile:
        nc.compile()
    return nc


def _get_nc():
    global _NC
    if _NC is None:
        _NC = build()
    return _NC


def _run(x, trace=False):
    from concourse.bass_utils import run_bass_kernel_spmd

    x = np.ascontiguousarray(np.asarray(x, dtype=np.float32))
    assert x.shape == (N, D)
    in_maps = [{"x": x, "xr": x[c * M:(c + 1) * M]} for c in range(NCORES)]
    res = run_bass_kernel_spmd(_get_nc(), in_maps, core_ids=list(range(NCORES)),
                               trace=trace)
    out = np.concatenate([res.results[c]["out"] for c in range(NCORES)], axis=0)
    return out, res


def kernel(x):
    out, _ = _run(x, trace=False)
    return out



# revision 11
# speedup vs baseline: 1.3331x; 1.3331x over previous
"""ContraNorm Trainium2 kernel (8 NeuronCores, fp8 DoubleRow, flash-style).

Reference computation (N=16384, D=256, f32):
    x_norm = x / max(||x||_row, 1e-12)
    sim    = softmax(x_norm @ x_norm.T, axis=1)
    out    = 1.1 * x - 0.1 * (sim @ x)

Sharding: row-parallel, no collectives. Core c receives x ROLLED so its own
2048 rows sit first; row-softmax and the P-accumulation are permutation
invariant over n, so rolling is transparent. Own rows = first M rows.

Per-core algorithm (all matmuls fp8e4 DoubleRow = 2x PE throughput; the two
128-row k-subtiles ride the byte-pair interleave produced by a uint16 DMA
transpose of packed fp8 pairs, so contraction index d = b*128 + p on both
operands automatically):
  setup (per 8-chunk group, pipelined with main):
    xt   = dma(x)                              f32 [128, 8, 256]
    xa   = fp8(x) pair-interleaved + [1,1]     [128, nch, 258] (gpsimd cast)
    ssq  = sum_d x^2 (ACT Square + accum), r16 = 16/||x|| = exp(-.5 ln ssq + ln16)
    xnT  = dma_transpose(xa chunk as u16)      compact [128, 2*n] fp8 raw x
    xnT *= r (broadcast over partitions)       => 16 * x_norm^T, fp8
  main, per chunk-pair and 512-row m-tile:
    S^T[n,m] = xnT_chunk.T @ xnT_band          (1 DoubleRow matmul per chunk)
    es = exp(S/256) -> fp8                     (ACT, [128,1024] instrs)
    Paug[m, 0:258] += es.T @ xa_pair           (DoubleRow; cols 256,257 = sumexp)
  finalize: out = 1.1*x_own - 0.1 * Pacc[:, perm]/Pacc[:, 256]
"""

import numpy as np

N, D, NCORES = 16384, 256, 8
M = N // NCORES          # 2048 rows per core
P = 128                  # partitions
SCALE = 0.1
LN16 = float(np.log(16.0))

_NC = None               # cached compiled Bass module


def build(n=N, m=M, compile=True):
    import concourse.bass as bass
    import concourse.tile as tile
    from concourse import bacc, mybir
    from contextlib import ExitStack

    F8 = mybir.dt.float8e4
    F16 = mybir.dt.float16
    F32 = mybir.dt.float32
    AF = mybir.ActivationFunctionType
    DR = mybir.MatmulPerfMode.DoubleRow

    nch = n // P             # n-chunks
    rch = m // P             # own row-chunks
    mt_w = min(512, m)       # m-tile width
    nmt = m // mt_w
    msub = mt_w // P
    TG = min(8, nch)         # chunks per setup group
    ngrp = nch // TG
    CP = min(16, nch)        # chunks per phase
    nph = nch // CP
    GPP = max(1, CP // TG)   # groups per phase
    LOOKAHEAD = 2

    # Keep Square/Ln/Exp on one ACT table set (avoids ~1.3us reloads).
    if not getattr(bacc, "_contranorm_act_patch", False):
        _orig_tables = bacc.get_activation_tables

        def _patched_tables(arch):
            keep = "natural_log_exp_and_others"
            return {k: (v if k == keep else set())
                    for k, v in _orig_tables(arch).items()}

        bacc.get_activation_tables = _patched_tables
        bacc._contranorm_act_patch = True

    nc = bacc.Bacc("TRN2", debug=False, num_devices=NCORES)
    x_d = nc.dram_tensor("x", (n, D), F32, kind="ExternalInput").ap()
    out_d = nc.dram_tensor("out", (m, D), F32, kind="ExternalOutput").ap()

    x_c = x_d.rearrange("(c p) d -> p c d", p=P)
    out_c = out_d.rearrange("(c p) d -> p c d", p=P)

    with tile.TileContext(nc) as tc, ExitStack() as ctx:
        big = ctx.enter_context(tc.tile_pool(name="big", bufs=1))
        ld = ctx.enter_context(tc.tile_pool(name="ld", bufs=2))
        ep = ctx.enter_context(tc.tile_pool(name="exp", bufs=3))
        fin = ctx.enter_context(tc.tile_pool(name="fin", bufs=2))
        sp = ctx.enter_context(tc.tile_pool(name="spsum", bufs=2, space="PSUM"))
        pp = ctx.enter_context(tc.tile_pool(name="ppsum", bufs=1, space="PSUM"))

        # persistent tiles
        xa = big.tile([P, nch, 258], F8)        # fp8 x, pair-interleaved + ones
        xnT = big.tile([P, 2, nch * P], F8)     # 16*x_norm^T, d-half planes
        xrs = big.tile([P, rch, D], F32)        # 1.1 * own rows (natural d order)
        pacc = big.tile([P, nmt * msub, 258], F32)
        ssq = big.tile([P, nch], F32)
        lnssq = big.tile([P, nch], F32)
        r32 = big.tile([P, nch], F32)           # 16/||x|| per row

        xts = {}
        xps = {}
        xtrs = {}

        def setup_piece(g, piece):
            c0 = g * TG
            if piece == 0:
                xt = xts[g] = ld.tile([P, TG, D], F32, name=f"xt{g}", tag="xt")
                nc.gpsimd.dma_start(xt[:], x_c[:, c0:c0 + TG, :])
                if c0 < rch:  # own band: keep 1.1*x for finalize
                    nc.vector.tensor_scalar_mul(
                        xrs[:, c0:c0 + TG, :], xt[:], 1.1)
            elif piece == 1:
                xt = xts[g]
                for j in range(TG):
                    c = c0 + j
                    # pack fp8 pairs: out col j*2+b <- d = b*128+j
                    nc.gpsimd.tensor_copy(
                        xa[:, c, 0:256].rearrange("p (j b) -> p b j", b=2),
                        xt[:, j, :].rearrange("p (b j) -> p b j", b=2))
                nc.gpsimd.memset(xa[:, c0:c0 + TG, 256:258], 1.0)
            elif piece == 2:
                xt = xts[g]
                scr = ld.tile([P, D], F16, tag="sq_scratch", name=f"scr{g}")
                for j in range(TG):
                    c = c0 + j
                    nc.scalar.activation(scr[:], xt[:, j, :], AF.Square,
                                         accum_out=ssq[:, c:c + 1])
            elif piece == 3:
                # r = 16/sqrt(ssq) = exp(-0.5 * ln(ssq/256))
                nc.scalar.activation(lnssq[:, c0:c0 + TG],
                                     ssq[:, c0:c0 + TG], AF.Ln,
                                     scale=1.0 / 256)
                nc.scalar.activation(r32[:, c0:c0 + TG],
                                     lnssq[:, c0:c0 + TG], AF.Exp,
                                     scale=-0.5)
            elif piece == 4:
                xt = xts.pop(g)
                xp = xps[g] = ld.tile([P, TG, 256], F8, name=f"xp{g}", tag="xp")
                for j in range(TG):
                    c = c0 + j
                    # 16*x_norm, fp8, pair-interleaved (per-partition scalar)
                    nc.vector.tensor_scalar_mul(
                        xp[:, j, :].rearrange("p (j2 b) -> p b j2", b=2),
                        xt[:, j, :].rearrange("p (b j2) -> p b j2", b=2),
                        r32[:, c:c + 1])
            elif piece == 5:
                xp = xps.pop(g)
                xtr = xtrs[g] = ld.tile([P, TG, 256], F8, name=f"xtr{g}",
                                        tag="xtr")
                for j in range(TG):
                    nc.sync.dma_start_transpose(
                        xtr[:, j, :].bitcast(F16),
                        xp[:, j, :].bitcast(F16))
            else:
                # de-interleave fp8 pairs into d-half planes of xnT
                xtr = xtrs.pop(g)
                src = xtr[:].rearrange("p c (j b) -> p b c j", b=2)
                band = slice(c0 * P, (c0 + TG) * P)
                nc.vector.tensor_copy(
                    xnT[:, 0, band].rearrange("p (c j) -> p c j", c=TG),
                    src[:, 0])
                nc.gpsimd.tensor_copy(
                    xnT[:, 1, band].rearrange("p (c j) -> p c j", c=TG),
                    src[:, 1])

        NPIECE = 7

        def setup_group(g):
            for piece in range(NPIECE):
                setup_piece(g, piece)

        def s_stationary(c):
            return xnT[:, :, c * P:(c + 1) * P]

        def s_moving(m0):
            return xnT[:, :, m0:m0 + mt_w]

        def phase(ph, setup_jobs=()):
            nonlocal pend
            jobs = list(setup_jobs)
            for mt in range(nmt):
                m0 = mt * mt_w
                paug = [pp.tile([P, 258], F32, tag=f"paug{ms}",
                                name=f"paug{ms}_{ph}_{mt}") for ms in range(msub)]
                for scn in range(CP // 2):
                    c0 = ph * CP + scn * 2
                    stp = sp.tile([P, 2, mt_w], F32, name=f"stp{ph}_{mt}_{scn}",
                                  tag="stp")
                    for j in range(2):
                        nc.tensor.matmul(stp[:, j, :], s_stationary(c0 + j),
                                         s_moving(m0), start=True, stop=True,
                                         perf_mode=DR)
                    es = ep.tile([P, 2, mt_w], F8, name=f"es{ph}_{mt}_{scn}",
                                 tag="es")
                    nc.scalar.activation(es[:], stp[:], AF.Exp, scale=1.0 / 256)
                    emit_pending()
                    pend = (es, c0, ph, paug, mt)
                nj = (len(jobs) + nmt - 1 - mt) // (nmt - mt) if jobs else 0
                for _ in range(nj):
                    g, piece = jobs.pop(0)
                    setup_piece(g, piece)

        def emit_pending():
            nonlocal pend
            if pend is None:
                return
            es, c0, ph, paug, mt = pend
            pend = None
            first = (c0 == ph * CP)
            last = (c0 == ph * CP + CP - 2)
            for ms in range(msub):
                nc.tensor.matmul(
                    paug[ms][:], es[:, :, ms * P:(ms + 1) * P],
                    xa[:, c0:c0 + 2, :], start=first, stop=last,
                    perf_mode=DR)
            if last:
                for ms in range(msub):
                    acc = pacc[:, mt * msub + ms, :]
                    if ph == 0:
                        nc.vector.tensor_copy(acc, paug[ms][:])
                    else:
                        nc.vector.tensor_add(acc, acc, paug[ms][:])

        def finalize():
            for rc in range(nmt * msub):
                r = fin.tile([P, 1], F32, tag="recip", name=f"r{rc}")
                nc.vector.reciprocal(r[:], pacc[:, rc, 256:257])
                rs = fin.tile([P, 1], F32, tag="rscaled", name=f"rs{rc}")
                nc.vector.tensor_scalar_mul(rs[:], r[:], -SCALE)
                t1 = fin.tile([P, D], F32, tag="scaledP", name=f"t1{rc}")
                # un-permute pair-interleaved cols: src (j*2+b) -> dst b*128+j
                nc.vector.tensor_scalar_mul(
                    t1[:].rearrange("p (b j) -> p b j", b=2),
                    pacc[:, rc, 0:256].rearrange("p (j b) -> p b j", b=2),
                    rs[:])
                ot = fin.tile([P, D], F32, tag="otile", name=f"ot{rc}")
                nc.vector.tensor_add(ot[:], xrs[:, rc, :], t1[:])
                nc.gpsimd.dma_start(out_c[:, rc, :], ot[:])

        pend = None
        prefill = min(ngrp, GPP * LOOKAHEAD)
        for g in range(prefill):
            setup_group(g)
        emitted = prefill
        for ph in range(nph):
            want = min(ngrp, GPP * (ph + 1 + LOOKAHEAD))
            jobs = [(g, piece) for g in range(emitted, want)
                    for piece in range(NPIECE)]
            emitted = want
            phase(ph, jobs)
        emit_pending()
        finalize()

    if compile:
        nc.compile()
    return nc


def _get_nc():
    global _NC
    if _NC is None:
        _NC = build()
    return _NC


def _run(x, trace=False):
    from concourse.bass_utils import run_bass_kernel_spmd

    x = np.ascontiguousarray(np.asarray(x, dtype=np.float32))
    assert x.shape == (N, D)
    in_maps = [{"x": np.ascontiguousarray(np.roll(x, -c * M, axis=0))}
               for c in range(NCORES)]
    res = run_bass_kernel_spmd(_get_nc(), in_maps, core_ids=list(range(NCORES)),
                               trace=trace)
    out = np.concatenate([res.results[c]["out"] for c in range(NCORES)], axis=0)
    return out, res


def kernel(x):
    out, _ = _run(x, trace=False)
    return out


# revision 15
# speedup vs baseline: 1.6422x; 1.2319x over previous
"""ContraNorm Trainium2 kernel (8 NeuronCores, fp8 DoubleRow, flash-style).

Reference computation (N=16384, D=256, f32):
    x_norm = x / max(||x||_row, 1e-12)
    sim    = softmax(x_norm @ x_norm.T, axis=1)
    out    = 1.1 * x - 0.1 * (sim @ x)

Sharding: row-parallel, no collectives. Core c receives x ROLLED so its own
2048 rows sit first; row-softmax and the P-accumulation are permutation
invariant over n, so rolling is transparent. Own rows = first M rows.

Per-core algorithm (all matmuls fp8e4 DoubleRow = 2x PE throughput; the two
128-row k-subtiles ride the byte-pair interleave produced by a uint16 DMA
transpose of packed fp8 pairs, so contraction index d = b*128 + p on both
operands automatically):
  setup (per 8-chunk group, pipelined with main):
    xt   = dma(x)                              f32 [128, 8, 256]
    xa   = fp8(x) pair-interleaved + [1,1]     [128, nch, 258] (gpsimd cast)
    ssq  = sum_d x^2 (ACT Square + accum), r16 = 16/||x|| = exp(-.5 ln ssq + ln16)
    xnT  = dma_transpose(xa chunk as u16)      compact [128, 2*n] fp8 raw x
    xnT *= r (broadcast over partitions)       => 16 * x_norm^T, fp8
  main, per chunk-pair and 512-row m-tile:
    S^T[n,m] = xnT_chunk.T @ xnT_band          (1 DoubleRow matmul per chunk)
    es = exp(S/256) -> fp8                     (ACT, [128,1024] instrs)
    Paug[m, 0:258] += es.T @ xa_pair           (DoubleRow; cols 256,257 = sumexp)
  finalize: out = 1.1*x_own - 0.1 * Pacc[:, perm]/Pacc[:, 256]
"""

import numpy as np

N, D, NCORES = 16384, 256, 8
M = N // NCORES          # 2048 rows per core
P = 128                  # partitions
SCALE = 0.1
LN16 = float(np.log(16.0))

_NC = None               # cached compiled Bass module


def build(n=N, m=M, compile=True):
    import concourse.bass as bass
    import concourse.tile as tile
    from concourse import bacc, mybir
    from contextlib import ExitStack

    F8 = mybir.dt.float8e4
    F16 = mybir.dt.float16
    F32 = mybir.dt.float32
    AF = mybir.ActivationFunctionType
    DR = mybir.MatmulPerfMode.DoubleRow

    nch = n // P             # n-chunks
    rch = m // P             # own row-chunks
    mt_w = min(512, m)       # m-tile width
    nmt = m // mt_w
    msub = mt_w // P
    TG = min(8, nch)         # chunks per setup group
    ngrp = nch // TG
    CP = min(16, nch)        # chunks per phase
    nph = nch // CP
    GPP = max(1, CP // TG)   # groups per phase
    LOOKAHEAD = 3

    # Keep Square/Ln/Exp on one ACT table set (avoids ~1.3us reloads).
    if not getattr(bacc, "_contranorm_act_patch", False):
        _orig_tables = bacc.get_activation_tables

        def _patched_tables(arch):
            keep = "natural_log_exp_and_others"
            return {k: (v if k == keep else set())
                    for k, v in _orig_tables(arch).items()}

        bacc.get_activation_tables = _patched_tables
        bacc._contranorm_act_patch = True

    nc = bacc.Bacc("TRN2", debug=False, num_devices=NCORES)
    x_d = nc.dram_tensor("x", (n, D), F32, kind="ExternalInput").ap()
    out_d = nc.dram_tensor("out", (m, D), F32, kind="ExternalOutput").ap()

    x_c = x_d.rearrange("(c p) d -> p c d", p=P)
    out_c = out_d.rearrange("(c p) d -> p c d", p=P)

    with tile.TileContext(nc) as tc, ExitStack() as ctx:
        big = ctx.enter_context(tc.tile_pool(name="big", bufs=1))
        ld = ctx.enter_context(tc.tile_pool(name="ld", bufs=2))
        ep = ctx.enter_context(tc.tile_pool(name="exp", bufs=3))
        fin = ctx.enter_context(tc.tile_pool(name="fin", bufs=2))
        sp = ctx.enter_context(tc.tile_pool(name="spsum", bufs=2, space="PSUM"))
        pp = ctx.enter_context(tc.tile_pool(name="ppsum", bufs=1, space="PSUM"))

        # persistent tiles
        xa = big.tile([P, nch, 258], F8)        # fp8 x, pair-interleaved + ones
        xnT = big.tile([P, 2, nch * P], F8)     # 16*x_norm^T, d-half planes
        xrs = big.tile([P, rch, D], F32)        # 1.1 * own rows (natural d order)
        pacc = big.tile([P, nmt * msub, 258], F32)
        ssq = big.tile([P, nch], F32)
        lnssq = big.tile([P, nch], F32)
        r32 = big.tile([P, nch], F32)           # 16/||x|| per row

        xts = {}
        xps = {}
        xtrs = {}

        def setup_piece(g, piece):
            c0 = g * TG
            if piece == 0:
                xt = xts[g] = ld.tile([P, TG, D], F32, name=f"xt{g}", tag="xt")
                nc.gpsimd.dma_start(xt[:], x_c[:, c0:c0 + TG, :])
                if c0 < rch:  # own band: keep 1.1*x for finalize
                    nc.vector.tensor_scalar_mul(
                        xrs[:, c0:c0 + TG, :], xt[:], 1.1)
            elif piece == 1:
                xt = xts[g]
                for j in range(TG):
                    c = c0 + j
                    # pack fp8 pairs: out col j*2+b <- d = b*128+j
                    nc.gpsimd.tensor_copy(
                        xa[:, c, 0:256].rearrange("p (j b) -> p b j", b=2),
                        xt[:, j, :].rearrange("p (b j) -> p b j", b=2))
                nc.gpsimd.memset(xa[:, c0:c0 + TG, 256:258], 1.0)
            elif piece == 2:
                xt = xts[g]
                scr = ld.tile([P, D], F16, tag="sq_scratch", name=f"scr{g}")
                for j in range(TG):
                    c = c0 + j
                    nc.scalar.activation(scr[:], xt[:, j, :], AF.Square,
                                         accum_out=ssq[:, c:c + 1])
            elif piece == 3:
                # r = 16/sqrt(ssq) = exp(-0.5 * ln(ssq/256))
                nc.scalar.activation(lnssq[:, c0:c0 + TG],
                                     ssq[:, c0:c0 + TG], AF.Ln,
                                     scale=1.0 / 256)
                nc.scalar.activation(r32[:, c0:c0 + TG],
                                     lnssq[:, c0:c0 + TG], AF.Exp,
                                     scale=-0.5)
            elif piece == 4:
                xt = xts.pop(g)
                xp = xps[g] = ld.tile([P, TG, 256], F8, name=f"xp{g}", tag="xp")
                for j in range(TG):
                    c = c0 + j
                    # 16*x_norm, fp8, pair-interleaved (per-partition scalar)
                    nc.vector.tensor_scalar_mul(
                        xp[:, j, :].rearrange("p (j2 b) -> p b j2", b=2),
                        xt[:, j, :].rearrange("p (b j2) -> p b j2", b=2),
                        r32[:, c:c + 1])
            elif piece == 5:
                xp = xps.pop(g)
                xtr = xtrs[g] = ld.tile([P, TG, 256], F8, name=f"xtr{g}",
                                        tag="xtr")
                nc.sync.dma_start_transpose(
                    xtr[:].bitcast(F16), xp[:].bitcast(F16))
            else:
                # de-interleave fp8 pairs into d-half planes of xnT
                xtr = xtrs.pop(g)
                src = xtr[:].rearrange("p c (j b) -> p b c j", b=2)
                band = slice(c0 * P, (c0 + TG) * P)
                for h in range(2):
                    nc.vector.tensor_copy(
                        xnT[:, h, band].rearrange("p (c j) -> p c j", c=TG),
                        src[:, h])

        NPIECE = 7

        def setup_group(g):
            for piece in range(NPIECE):
                setup_piece(g, piece)

        def s_stationary(c):
            return xnT[:, :, c * P:(c + 1) * P]

        def s_moving(m0):
            return xnT[:, :, m0:m0 + mt_w]

        def phase(ph, setup_jobs=()):
            nonlocal pend
            jobs = list(setup_jobs)
            for mt in range(nmt):
                m0 = mt * mt_w
                paug = [pp.tile([P, 258], F32, tag=f"paug{ms}",
                                name=f"paug{ms}_{ph}_{mt}") for ms in range(msub)]
                for scn in range(CP // 2):
                    c0 = ph * CP + scn * 2
                    stp = sp.tile([P, 2, mt_w], F32, name=f"stp{ph}_{mt}_{scn}",
                                  tag="stp")
                    for j in range(2):
                        nc.tensor.matmul(stp[:, j, :], s_stationary(c0 + j),
                                         s_moving(m0), start=True, stop=True,
                                         perf_mode=DR)
                    es = ep.tile([P, 2, mt_w], F8, name=f"es{ph}_{mt}_{scn}",
                                 tag="es")
                    nc.scalar.activation(es[:], stp[:], AF.Exp, scale=1.0 / 256)
                    emit_pending()
                    pend = (es, c0, ph, paug, mt)
                nj = (len(jobs) + nmt - 1 - mt) // (nmt - mt) if jobs else 0
                for _ in range(nj):
                    g, piece = jobs.pop(0)
                    setup_piece(g, piece)

        def emit_pending():
            nonlocal pend
            if pend is None:
                return
            es, c0, ph, paug, mt = pend
            pend = None
            first = (c0 == ph * CP)
            last = (c0 == ph * CP + CP - 2)
            for ms in range(msub):
                nc.tensor.matmul(
                    paug[ms][:], es[:, :, ms * P:(ms + 1) * P],
                    xa[:, c0:c0 + 2, :], start=first, stop=last,
                    perf_mode=DR)
            if last:
                for ms in range(msub):
                    acc = pacc[:, mt * msub + ms, :]
                    if ph == 0:
                        nc.vector.tensor_copy(acc, paug[ms][:])
                    else:
                        nc.vector.tensor_add(acc, acc, paug[ms][:])

        def finalize():
            for rc in range(nmt * msub):
                r = fin.tile([P, 1], F32, tag="recip", name=f"r{rc}")
                nc.vector.reciprocal(r[:], pacc[:, rc, 256:257])
                rs = fin.tile([P, 1], F32, tag="rscaled", name=f"rs{rc}")
                nc.vector.tensor_scalar_mul(rs[:], r[:], -SCALE)
                t1 = fin.tile([P, D], F32, tag="scaledP", name=f"t1{rc}")
                # un-permute pair-interleaved cols: src (j*2+b) -> dst b*128+j
                nc.vector.tensor_scalar_mul(
                    t1[:].rearrange("p (b j) -> p b j", b=2),
                    pacc[:, rc, 0:256].rearrange("p (j b) -> p b j", b=2),
                    rs[:])
                ot = fin.tile([P, D], F32, tag="otile", name=f"ot{rc}")
                nc.vector.tensor_add(ot[:], xrs[:, rc, :], t1[:])
                nc.gpsimd.dma_start(out_c[:, rc, :], ot[:])

        pend = None
        prefill = min(ngrp, GPP * LOOKAHEAD)
        for g in range(prefill):
            setup_group(g)
        emitted = prefill
        for ph in range(nph):
            want = min(ngrp, GPP * (ph + 1 + LOOKAHEAD))
            jobs = [(g, piece) for g in range(emitted, want)
                    for piece in range(NPIECE)]
            emitted = want
            phase(ph, jobs)
        emit_pending()
        finalize()

    if compile:
        nc.compile()
    return nc


def _get_nc():
    global _NC
    if _NC is None:
        _NC = build()
    return _NC


def _run(x, trace=False):
    from concourse.bass_utils import run_bass_kernel_spmd

    x = np.ascontiguousarray(np.asarray(x, dtype=np.float32))
    assert x.shape == (N, D)
    in_maps = [{"x": np.ascontiguousarray(np.roll(x, -c * M, axis=0))}
               for c in range(NCORES)]
    res = run_bass_kernel_spmd(_get_nc(), in_maps, core_ids=list(range(NCORES)),
                               trace=trace)
    out = np.concatenate([res.results[c]["out"] for c in range(NCORES)], axis=0)
    return out, res


def kernel(x):
    out, _ = _run(x, trace=False)
    return out
